# revision 9
# baseline (speedup 1.0000x reference)
"""Trainium2 Bass kernel for nn_Attention2 (sparse additive attention).

Math (per batch b):
    att_h  = h @ W_h2att.T + b_h2att                       [HID]
    dot    = tanh(p_att_feats[b] + att_h)                  [S, HID]
    scores = dot @ w_alpha (+ b_alpha, cancels in softmax) [S]
    scores = where(mask, -1e8, scores)
    w      = softmax(scores)          (masked rows get weight exactly 0)
    out[b] = w @ att_feats[b]                              [RNN]

Strategy: pure data parallel over batch (16 batches / core on 8 cores).
Rows (b, s) with mask==True contribute exactly zero (exp(-1e8) == 0 in
fp32), so the kernel gathers only unmasked rows (indirect DMA), packs
them densely into 128-row chunks that may span batches, and uses
host-built one-hot batch matrices to run segmented reductions on the
tensor engine:
    bc   = oht.T @ att_h        (per-row broadcast of its batch's att_h)
    psum = bc + I @ p_rows      (the broadcast-add, done on the PE)
    t    = tanh(psum)                               [ACT]
    s    = reduce(t * w_alpha_bcast)                [DVE, fused ttr]
    e    = exp(s)                                   [ACT]
    ohw  = oh * e                                   [DVE]
    res  += ohw.T @ A_rows ; sums += ohw.T @ ones   [PE, psum accum]
    out  = res / sums
b_alpha and the softmax max-subtraction cancel and are omitted.
"""

import os
import sys
from contextlib import ExitStack

import numpy as np

for _p in (
    "/root/.axon_site",
    "/root/.axon_site/_ro/trn_rl_repo",
    "/root/.axon_site/_ro/pypackages",
    "/opt/trn_rl_repo",
):
    if os.path.isdir(_p) and _p not in sys.path:
        sys.path.append(_p)

import concourse.bass as bass
import concourse.tile as tile
from concourse import bacc, mybir
from concourse.bass import IndirectOffsetOnAxis
from concourse.bass_utils import run_bass_kernel_spmd

B, S, RNN, HID = 128, 1024, 1024, 512
NCORES = 8
BS = B // NCORES  # batches per core
P = 128
F32 = mybir.dt.float32
F32R = mybir.dt.float32r
I32 = mybir.dt.int32
KCHUNKS = RNN // P  # k-chunks for the att_h matmul


def _build_program(nchunks: int, use_gather: bool, use_f32r: bool):
    nc = bacc.Bacc("TRN2", target_bir_lowering=False, debug=False, num_devices=NCORES)

    # Tensors consumed as matmul operands are declared float32r when
    # use_f32r: same bits/bytes host-side (np.float32), but the PE streams
    # them in a single pass (4x faster than fp32's 2x half-speed passes).
    mmdt = F32R if use_f32r else F32

    A_d = nc.dram_tensor("A", [BS * S, RNN], mmdt, kind="ExternalInput").ap()
    p_d = nc.dram_tensor("p", [BS * S, HID], mmdt, kind="ExternalInput").ap()
    hT_d = nc.dram_tensor("hT", [RNN, BS], F32, kind="ExternalInput").ap()
    wT_d = nc.dram_tensor("wT", [RNN, HID], F32, kind="ExternalInput").ap()
    bias_d = nc.dram_tensor("bias", [1, HID], F32, kind="ExternalInput").ap()
    wab_d = nc.dram_tensor("wab", [P, HID], F32, kind="ExternalInput").ap()
    oh_d = nc.dram_tensor("oh", [nchunks, P, BS], F32, kind="ExternalInput").ap()
    oht_d = nc.dram_tensor("oht", [nchunks, BS, P], mmdt, kind="ExternalInput").ap()
    ident_d = nc.dram_tensor("ident", [P, P], mmdt, kind="ExternalInput").ap()
    ones_d = nc.dram_tensor("ones", [P, 8], mmdt, kind="ExternalInput").ap()
    if use_gather:
        idx_d = nc.dram_tensor("idx", [nchunks, P], I32, kind="ExternalInput").ap()
    out_d = nc.dram_tensor("out", [BS, RNN], F32, kind="ExternalOutput").ap()

    with tile.TileContext(nc) as tc, ExitStack() as ctx:
        const = ctx.enter_context(tc.tile_pool(name="const", bufs=1))
        loads = ctx.enter_context(tc.tile_pool(name="loads", bufs=4))
        meta = ctx.enter_context(tc.tile_pool(name="meta", bufs=4))
        work = ctx.enter_context(tc.tile_pool(name="work", bufs=3))
        small = ctx.enter_context(tc.tile_pool(name="small", bufs=3))
        ps_work = ctx.enter_context(tc.tile_pool(name="ps_work", bufs=3, space="PSUM"))
        ps_hold = ctx.enter_context(tc.tile_pool(name="ps_hold", bufs=1, space="PSUM"))

        # ---- setup: constants + att_h = h @ W.T + bias ----
        wT_sb = const.tile([P, KCHUNKS, HID], F32)
        nc.sync.dma_start(out=wT_sb, in_=wT_d.rearrange("(j p) d -> p j d", p=P))
        hT_sb = const.tile([P, KCHUNKS, BS], F32)
        nc.sync.dma_start(out=hT_sb, in_=hT_d.rearrange("(j p) b -> p j b", p=P))
        bias_sb = const.tile([1, HID], F32)
        nc.sync.dma_start(out=bias_sb, in_=bias_d)
        wab_sb = const.tile([P, HID], F32)
        nc.sync.dma_start(out=wab_sb, in_=wab_d)
        ident_sb = const.tile([P, P], mmdt)
        nc.sync.dma_start(out=ident_sb, in_=ident_d)
        ones_sb = const.tile([P, 8], mmdt)
        nc.sync.dma_start(out=ones_sb, in_=ones_d)
        onesb_sb = const.tile([1, BS], F32)
        nc.vector.memset(onesb_sb, 1.0)

        att_ps = ps_hold.tile([BS, HID], F32)
        for j in range(KCHUNKS):
            nc.tensor.matmul(
                out=att_ps,
                lhsT=hT_sb[:, j, :],
                rhs=wT_sb[:, j, :],
                start=(j == 0),
                stop=False,
            )
        nc.tensor.matmul(out=att_ps, lhsT=onesb_sb, rhs=bias_sb, start=False, stop=True)
        att_h_sb = const.tile([BS, HID], mmdt)
        nc.scalar.copy(att_h_sb, att_ps)

        res_ps = ps_hold.tile([BS, RNN], F32)
        sums_ps = ps_hold.tile([BS, 8], F32)

        # ---- main loop over packed 128-row chunks ----
        for c in range(nchunks):
            oh_sb = meta.tile([P, BS], F32)
            nc.sync.dma_start(out=oh_sb, in_=oh_d[c])
            oht_sb = meta.tile([BS, P], mmdt)
            nc.sync.dma_start(out=oht_sb, in_=oht_d[c])

            p_t = loads.tile([P, HID], mmdt, tag="p_t")
            A_t = loads.tile([P, RNN], mmdt, tag="A_t")
            if use_gather:
                idx_sb = meta.tile([P, 1], I32)
                nc.sync.dma_start(out=idx_sb, in_=idx_d[c].unsqueeze(1))
                nc.gpsimd.indirect_dma_start(
                    out=p_t,
                    out_offset=None,
                    in_=p_d,
                    in_offset=IndirectOffsetOnAxis(ap=idx_sb, axis=0),
                )
                nc.gpsimd.indirect_dma_start(
                    out=A_t,
                    out_offset=None,
                    in_=A_d,
                    in_offset=IndirectOffsetOnAxis(ap=idx_sb, axis=0),
                )
            else:
                nc.sync.dma_start(out=p_t, in_=p_d[c * P : (c + 1) * P, :])
                nc.sync.dma_start(out=A_t, in_=A_d[c * P : (c + 1) * P, :])

            # psum = oht.T @ att_h + I @ p  (broadcast att_h row + add p)
            w_ps = ps_work.tile([P, HID], F32)
            nc.tensor.matmul(
                out=w_ps, lhsT=oht_sb, rhs=att_h_sb, start=True, stop=False
            )
            nc.tensor.matmul(
                out=w_ps, lhsT=ident_sb, rhs=p_t, start=False, stop=True
            )

            tanh_t = work.tile([P, HID], F32, tag="tanh")
            nc.scalar.activation(
                out=tanh_t, in_=w_ps, func=mybir.ActivationFunctionType.Tanh
            )

            # scores col = sum(tanh * w_alpha): DVE multiply, then ACT
            # Copy-with-accum does the free-dim reduction (ttr is broken on HW).
            scr = work.tile([P, HID], F32, tag="scr")
            nc.vector.tensor_mul(scr, tanh_t, wab_sb)
            sc_col = small.tile([P, 1], F32, tag="sc")
            dump = work.tile([P, HID], F32, tag="dump")
            nc.scalar.activation(
                out=dump,
                in_=scr,
                func=mybir.ActivationFunctionType.Copy,
                accum_out=sc_col,
            )

            exp_t = small.tile([P, 1], F32, tag="exp")
            nc.scalar.activation(
                out=exp_t, in_=sc_col, func=mybir.ActivationFunctionType.Exp
            )

            ohw = small.tile([P, BS], mmdt, tag="ohw")
            nc.vector.tensor_scalar_mul(out=ohw, in0=oh_sb, scalar1=exp_t)

            st, sp = (c == 0), (c == nchunks - 1)
            nc.tensor.matmul(
                out=res_ps[:, 0:512],
                lhsT=ohw,
                rhs=A_t[:, 0:512],
                start=st,
                stop=sp,
            )
            nc.tensor.matmul(
                out=res_ps[:, 512:1024],
                lhsT=ohw,
                rhs=A_t[:, 512:1024],
                start=st,
                stop=sp,
            )
            nc.tensor.matmul(
                out=sums_ps, lhsT=ohw, rhs=ones_sb, start=st, stop=sp
            )

        # ---- normalize + store ----
        recip_sb = const.tile([BS, 1], F32)
        nc.vector.reciprocal(recip_sb, sums_ps[:, 0:1])
        out_sb = const.tile([BS, RNN], F32)
        nc.vector.tensor_scalar_mul(out=out_sb, in0=res_ps, scalar1=recip_sb)
        nc.sync.dma_start(out=out_d, in_=out_sb)

    nc.compile()
    return nc


def _prep_core(m, h, A, p, mask, use_gather):
    """Host-side shard prep for core m. Returns (in_map, nchunks_needed)."""
    sl = slice(m * BS, (m + 1) * BS)
    mask_m = mask[sl]
    if use_gather:
        rows = np.concatenate(
            [b * S + np.flatnonzero(~mask_m[b]) for b in range(BS)]
        ).astype(np.int64)
    else:
        rows = np.arange(BS * S, dtype=np.int64)
    r = len(rows)
    nch = (r + P - 1) // P
    in_map = {
        "A": np.ascontiguousarray(A[sl].reshape(BS * S, RNN)),
        "p": np.ascontiguousarray(p[sl].reshape(BS * S, HID)),
        "hT": np.ascontiguousarray(h[sl].T),
    }
    return in_map, rows, nch


def _finish_core(in_map, rows, nchunks, mask_flat_m, use_gather):
    """Pad metadata to the common nchunks and add oh/oht/idx."""
    r = len(rows)
    total = nchunks * P
    idx = np.zeros(total, np.int32)
    idx[:r] = rows
    oh = np.zeros((total, BS), np.float32)
    if use_gather:
        oh[np.arange(r), (rows // S).astype(np.int64)] = 1.0
    else:
        keep = ~mask_flat_m  # [BS*S] — masked rows get all-zero one-hot rows
        kk = np.flatnonzero(keep)
        oh[kk, (kk // S).astype(np.int64)] = 1.0
    oh = oh.reshape(nchunks, P, BS)
    oht = np.ascontiguousarray(oh.transpose(0, 2, 1))
    in_map["oh"] = oh
    in_map["oht"] = oht
    if use_gather:
        in_map["idx"] = idx.reshape(nchunks, P)
    return in_map


def run(
    inputs,
    use_gather: bool = True,
    use_f32r: bool = True,
    trace: bool = False,
    trace_kwargs: dict | None = None,
):
    h = np.asarray(inputs["h"], dtype=np.float32)
    A = np.asarray(inputs["att_feats"], dtype=np.float32)
    p = np.asarray(inputs["p_att_feats"], dtype=np.float32)
    mask = np.asarray(inputs["mask"]).astype(bool)
    W = np.asarray(inputs["W_h2att"], dtype=np.float32)
    bh = np.asarray(inputs["b_h2att"], dtype=np.float32)
    wa = np.asarray(inputs["w_alpha"], dtype=np.float32)

    shared = {
        "wT": np.ascontiguousarray(W.T),
        "bias": np.ascontiguousarray(bh[None, :]),
        "wab": np.ascontiguousarray(np.broadcast_to(wa[None, :], (P, HID))),
        "ident": np.eye(P, dtype=np.float32),
        "ones": np.ones((P, 8), np.float32),
    }

    per_core = [_prep_core(m, h, A, p, mask, use_gather) for m in range(NCORES)]
    nchunks = max(nch for (_, _, nch) in per_core)
    in_maps = []
    for m, (in_map, rows, _) in enumerate(per_core):
        mask_flat = mask[m * BS : (m + 1) * BS].reshape(-1)
        in_map = _finish_core(in_map, rows, nchunks, mask_flat, use_gather)
        in_map.update(shared)
        in_maps.append(in_map)

    nc = _build_program(nchunks, use_gather, use_f32r)
    br = run_bass_kernel_spmd(
        nc,
        in_maps,
        core_ids=list(range(NCORES)),
        trace=trace,
        **(trace_kwargs or {}),
    )
    out = np.concatenate([br.results[m]["out"] for m in range(NCORES)], axis=0)
    return out.astype(np.float32), br


def kernel(**inputs) -> np.ndarray:
    use_gather = os.environ.get("ATT_USE_GATHER", "1") == "1"
    use_f32r = os.environ.get("ATT_USE_F32R", "1") == "1"
    out, _ = run(inputs, use_gather=use_gather, use_f32r=use_f32r, trace=False)
    return out


# revision 14
# speedup vs baseline: 1.5133x; 1.5133x over previous
"""Trainium2 Bass kernel for nn_Attention2 (sparse additive attention).

Math (per batch b):
    att_h  = h @ W_h2att.T + b_h2att                       [HID]
    dot    = tanh(p_att_feats[b] + att_h)                  [S, HID]
    scores = dot @ w_alpha (+ b_alpha, cancels in softmax) [S]
    scores = where(mask, -1e8, scores)
    w      = softmax(scores)          (masked rows get weight exactly 0)
    out[b] = w @ att_feats[b]                              [RNN]

Strategy: pure data parallel over batch (16 batches / core on 8 cores).
Rows (b, s) with mask==True contribute exactly zero (exp(-1e8) == 0 in
fp32), so the kernel gathers only unmasked rows (one indirect DMA per
128-row chunk over host-fused [p | A] rows), packs them densely into
chunks that may span batches, and uses host-built one-hot batch
matrices for segmented reductions on the tensor engine:
    bc   = oht.T @ att_h       (per-row broadcast of its batch's att_h)
    x    = bc + p                                   [DVE]
    t    = tanh(x)                                  [ACT]
    s    = reduce(t * w_alpha_bcast)                [DVE mul + reduce]
    e    = exp(s)                                   [ACT]
    ohw  = oh * e                                   [DVE]
    res  += ohw.T @ A_rows ; sums += ohw.T @ ones   [PE, psum accum]
    out  = res / sums
b_alpha and the softmax max-subtraction cancel and are omitted.

dt_mode: "f32" (exact), "f32r" (PE streams fp32 in 1 pass, ~1e-4 err),
"bf16" (gathered data + matmul operands in bf16, ~5e-4 err, halves DMA).
"""

import os
import sys
from contextlib import ExitStack

import numpy as np

for _p in (
    "/root/.axon_site",
    "/root/.axon_site/_ro/trn_rl_repo",
    "/root/.axon_site/_ro/pypackages",
    "/opt/trn_rl_repo",
):
    if os.path.isdir(_p) and _p not in sys.path:
        sys.path.append(_p)

import ml_dtypes
import concourse.bass as bass
import concourse.tile as tile
from concourse import bacc, mybir
from concourse.bass import IndirectOffsetOnAxis
from concourse.bass_utils import run_bass_kernel_spmd

B, S, RNN, HID = 128, 1024, 1024, 512
NCORES = 8
BS = B // NCORES  # batches per core
P = 128
F32 = mybir.dt.float32
F32R = mybir.dt.float32r
BF16 = mybir.dt.bfloat16
I32 = mybir.dt.int32
KCHUNKS = RNN // P  # k-chunks for the att_h matmul

_DT = {"f32": F32, "f32r": F32R, "bf16": BF16}
_NPDT = {"f32": np.float32, "f32r": np.float32, "bf16": ml_dtypes.bfloat16}


def _build_program(nchunks: int, use_gather: bool, dt_mode: str):
    nc = bacc.Bacc("TRN2", target_bir_lowering=False, debug=False, num_devices=NCORES)
    mmdt = _DT[dt_mode]
    bf = dt_mode == "bf16"

    pA_d = nc.dram_tensor("pA", [BS * S, HID + RNN], mmdt, kind="ExternalInput").ap()
    hT_d = nc.dram_tensor("hT", [RNN, BS], F32, kind="ExternalInput").ap()
    wT_d = nc.dram_tensor("wT", [RNN, HID], F32, kind="ExternalInput").ap()
    bias_d = nc.dram_tensor("bias", [1, HID], F32, kind="ExternalInput").ap()
    wab_d = nc.dram_tensor("wab", [P, HID], mmdt, kind="ExternalInput").ap()
    oh_d = nc.dram_tensor("oh", [nchunks, P, BS], mmdt, kind="ExternalInput").ap()
    oht_d = nc.dram_tensor("oht", [nchunks, BS, P], mmdt, kind="ExternalInput").ap()
    ones_d = nc.dram_tensor("ones", [P, 8], mmdt, kind="ExternalInput").ap()
    if use_gather:
        idx_d = nc.dram_tensor("idx", [nchunks, P], I32, kind="ExternalInput").ap()
    out_d = nc.dram_tensor("out", [BS, RNN], F32, kind="ExternalOutput").ap()

    with tile.TileContext(nc) as tc, ExitStack() as ctx:
        const = ctx.enter_context(tc.tile_pool(name="const", bufs=1))
        loads = ctx.enter_context(tc.tile_pool(name="loads", bufs=6))
        work = ctx.enter_context(tc.tile_pool(name="work", bufs=4))
        small = ctx.enter_context(tc.tile_pool(name="small", bufs=6))
        ps_work = ctx.enter_context(tc.tile_pool(name="ps_work", bufs=4, space="PSUM"))
        ps_hold = ctx.enter_context(tc.tile_pool(name="ps_hold", bufs=1, space="PSUM"))

        # ---- setup: constants + att_h = h @ W.T + bias ----
        wT_sb = const.tile([P, KCHUNKS, HID], F32)
        nc.sync.dma_start(out=wT_sb, in_=wT_d.rearrange("(j p) d -> p j d", p=P))
        hT_sb = const.tile([P, KCHUNKS, BS], F32)
        nc.sync.dma_start(out=hT_sb, in_=hT_d.rearrange("(j p) b -> p j b", p=P))
        bias_sb = const.tile([1, HID], F32)
        nc.sync.dma_start(out=bias_sb, in_=bias_d)
        wab_sb = const.tile([P, HID], mmdt)
        nc.sync.dma_start(out=wab_sb, in_=wab_d)
        ones_sb = const.tile([P, 8], mmdt)
        nc.sync.dma_start(out=ones_sb, in_=ones_d)
        onesb_sb = const.tile([1, BS], F32)
        nc.vector.memset(onesb_sb, 1.0)

        # bulk-load all per-chunk metadata once
        oh_all = const.tile([P, nchunks, BS], mmdt)
        nc.sync.dma_start(out=oh_all, in_=oh_d.rearrange("c p b -> p c b"))
        oht_all = const.tile([BS, nchunks, P], mmdt)
        nc.sync.dma_start(out=oht_all, in_=oht_d.rearrange("c b p -> b c p"))
        if use_gather:
            idx_all = const.tile([P, nchunks], I32)
            nc.sync.dma_start(out=idx_all, in_=idx_d.rearrange("c p -> p c"))

        att_ps = ps_hold.tile([BS, HID], F32)
        for j in range(KCHUNKS):
            nc.tensor.matmul(
                out=att_ps,
                lhsT=hT_sb[:, j, :],
                rhs=wT_sb[:, j, :],
                start=(j == 0),
                stop=False,
            )
        nc.tensor.matmul(out=att_ps, lhsT=onesb_sb, rhs=bias_sb, start=False, stop=True)
        att_h_sb = const.tile([BS, HID], mmdt)
        nc.scalar.copy(att_h_sb, att_ps)

        res_ps = ps_hold.tile([BS, RNN], F32)
        sums_ps = ps_hold.tile([BS, 8], F32)

        # ---- main loop over packed 128-row chunks ----
        for c in range(nchunks):
            oh_sb = oh_all[:, c, :]
            oht_sb = oht_all[:, c, :]

            pA_t = loads.tile([P, HID + RNN], mmdt, tag="pA_t")
            if use_gather:
                nc.gpsimd.indirect_dma_start(
                    out=pA_t,
                    out_offset=None,
                    in_=pA_d,
                    in_offset=IndirectOffsetOnAxis(ap=idx_all[:, c : c + 1], axis=0),
                )
            else:
                nc.sync.dma_start(out=pA_t, in_=pA_d[c * P : (c + 1) * P, :])
            p_t = pA_t[:, 0:HID]
            A_t = pA_t[:, HID : HID + RNN]

            # bc = oht.T @ att_h (row-wise broadcast of att_h), add p on DVE
            w_ps = ps_work.tile([P, HID], F32)
            nc.tensor.matmul(out=w_ps, lhsT=oht_sb, rhs=att_h_sb, start=True, stop=True)
            x_t = work.tile([P, HID], F32, tag="x")
            nc.vector.tensor_add(x_t, w_ps, p_t)

            tanh_t = work.tile([P, HID], mmdt if bf else F32, tag="tanh")
            nc.scalar.activation(
                out=tanh_t, in_=x_t, func=mybir.ActivationFunctionType.Tanh
            )

            # scores col = sum(tanh * w_alpha)
            scr = work.tile([P, HID], mmdt if bf else F32, tag="scr")
            nc.vector.tensor_mul(scr, tanh_t, wab_sb)
            sc_col = small.tile([P, 1], F32, tag="sc")
            if bf:
                nc.vector.tensor_reduce(
                    out=sc_col,
                    in_=scr,
                    axis=mybir.AxisListType.X,
                    op=mybir.AluOpType.add,
                )
            else:
                dump = work.tile([P, HID], F32, tag="dump")
                nc.scalar.activation(
                    out=dump,
                    in_=scr,
                    func=mybir.ActivationFunctionType.Copy,
                    accum_out=sc_col,
                )

            exp_t = small.tile([P, 1], F32, tag="exp")
            nc.scalar.activation(
                out=exp_t, in_=sc_col, func=mybir.ActivationFunctionType.Exp
            )

            ohw = small.tile([P, BS], mmdt, tag="ohw")
            nc.vector.tensor_scalar_mul(out=ohw, in0=oh_sb, scalar1=exp_t)

            st, sp = (c == 0), (c == nchunks - 1)
            nc.tensor.matmul(
                out=res_ps[:, 0:512],
                lhsT=ohw,
                rhs=A_t[:, 0:512],
                start=st,
                stop=sp,
            )
            nc.tensor.matmul(
                out=res_ps[:, 512:1024],
                lhsT=ohw,
                rhs=A_t[:, 512:1024],
                start=st,
                stop=sp,
            )
            nc.tensor.matmul(out=sums_ps, lhsT=ohw, rhs=ones_sb, start=st, stop=sp)

        # ---- normalize + store ----
        recip_sb = const.tile([BS, 1], F32)
        nc.vector.reciprocal(recip_sb, sums_ps[:, 0:1])
        out_sb = const.tile([BS, RNN], F32)
        nc.vector.tensor_scalar_mul(out=out_sb, in0=res_ps, scalar1=recip_sb)
        nc.sync.dma_start(out=out_d, in_=out_sb)

    nc.compile()
    return nc


def _prep_core(m, h, pA_full, mask, use_gather, npdt):
    """Host-side shard prep for core m."""
    sl = slice(m * BS, (m + 1) * BS)
    mask_m = mask[sl]
    if use_gather:
        rows = np.concatenate(
            [b * S + np.flatnonzero(~mask_m[b]) for b in range(BS)]
        ).astype(np.int64)
    else:
        rows = np.arange(BS * S, dtype=np.int64)
    nch = (len(rows) + P - 1) // P
    in_map = {
        "pA": pA_full[m],
        "hT": np.ascontiguousarray(h[sl].T),
    }
    return in_map, rows, nch


def _finish_core(in_map, rows, nchunks, mask_flat_m, use_gather, npdt):
    r = len(rows)
    total = nchunks * P
    idx = np.zeros(total, np.int32)
    idx[:r] = rows
    oh = np.zeros((total, BS), np.float32)
    if use_gather:
        oh[np.arange(r), (rows // S).astype(np.int64)] = 1.0
    else:
        keep = ~mask_flat_m
        kk = np.flatnonzero(keep)
        oh[kk, (kk // S).astype(np.int64)] = 1.0
    oh = oh.reshape(nchunks, P, BS)
    oht = np.ascontiguousarray(oh.transpose(0, 2, 1))
    in_map["oh"] = oh.astype(npdt)
    in_map["oht"] = oht.astype(npdt)
    if use_gather:
        in_map["idx"] = idx.reshape(nchunks, P)
    return in_map


def run(
    inputs,
    use_gather: bool = True,
    dt_mode: str = "bf16",
    trace: bool = False,
    trace_kwargs: dict | None = None,
):
    h = np.asarray(inputs["h"], dtype=np.float32)
    A = np.asarray(inputs["att_feats"], dtype=np.float32)
    p = np.asarray(inputs["p_att_feats"], dtype=np.float32)
    mask = np.asarray(inputs["mask"]).astype(bool)
    W = np.asarray(inputs["W_h2att"], dtype=np.float32)
    bh = np.asarray(inputs["b_h2att"], dtype=np.float32)
    wa = np.asarray(inputs["w_alpha"], dtype=np.float32)
    npdt = _NPDT[dt_mode]

    # fused [p | A] rows per core, in the matmul dtype
    pA_full = []
    for m in range(NCORES):
        sl = slice(m * BS, (m + 1) * BS)
        pA = np.empty((BS * S, HID + RNN), npdt)
        pA[:, :HID] = p[sl].reshape(BS * S, HID).astype(npdt)
        pA[:, HID:] = A[sl].reshape(BS * S, RNN).astype(npdt)
        pA_full.append(pA)

    shared = {
        "wT": np.ascontiguousarray(W.T),
        "bias": np.ascontiguousarray(bh[None, :]),
        "wab": np.ascontiguousarray(
            np.broadcast_to(wa[None, :], (P, HID)).astype(npdt)
        ),
        "ones": np.ones((P, 8), npdt),
    }

    per_core = [
        _prep_core(m, h, pA_full, mask, use_gather, npdt) for m in range(NCORES)
    ]
    nchunks = max(nch for (_, _, nch) in per_core)
    in_maps = []
    for m, (in_map, rows, _) in enumerate(per_core):
        mask_flat = mask[m * BS : (m + 1) * BS].reshape(-1)
        in_map = _finish_core(in_map, rows, nchunks, mask_flat, use_gather, npdt)
        in_map.update(shared)
        in_maps.append(in_map)

    nc = _build_program(nchunks, use_gather, dt_mode)
    br = run_bass_kernel_spmd(
        nc,
        in_maps,
        core_ids=list(range(NCORES)),
        trace=trace,
        **(trace_kwargs or {}),
    )
    out = np.concatenate([br.results[m]["out"] for m in range(NCORES)], axis=0)
    return out.astype(np.float32), br


def kernel(**inputs) -> np.ndarray:
    use_gather = os.environ.get("ATT_USE_GATHER", "1") == "1"
    dt_mode = os.environ.get("ATT_DT_MODE", "bf16")
    out, _ = run(inputs, use_gather=use_gather, dt_mode=dt_mode, trace=False)
    return out
